# revision 19
# baseline (speedup 1.0000x reference)
"""TGCN (GCNConv + GRUCell) Bass kernel for 8 TRN2 NeuronCores.

Strategy: shard nodes (dst) across 8 cores; each core owns N/8 = 6250 dst nodes.
Edges (incl. synthesized GCN self-loops) are destination-sorted and bucketed by
(128-dst-block, src-half). Each core gathers the x-rows of its edges' sources
straight from its own HBM copy of x via dma_gather (512B rows, int16 indices —
hence the src-half split at row 32768), builds a one-hot-times-norm scatter
matrix S on the vector engine (iota == dst_in_block - window_start; buckets are
dst-sorted so a 128-edge tile's dst range almost always fits a 32-wide window),
and accumulates agg^T[feat, dst] = sum_e norm_e * x[src_e]^T via PSUM-accumulated
PE matmuls. GCN linear layer + GRU run node-local on the shard, feature-major.
"""
import sys

sys.path.insert(0, '/opt/trn_rl_repo')

import numpy as np

N = 50000
E_IN = 800000
IN_C = 128
HID = 64
CORES = 8
SHARD = N // CORES          # 6250
P = 128
BLOCKS = (SHARD + P - 1) // P   # 49
HALF = 32768                # int16-safe gather index limit
GROUP_BLOCKS = 2
WNARROW = 32                # narrow one-hot window width
PADC = BLOCKS * P           # 6272 padded shard width

last_nc = None              # populated at build time, for test tooling


class Plan:
    pass


def _host_prep(x, edge_index, edge_weight):
    src = np.asarray(edge_index[0], dtype=np.int64)
    dst = np.asarray(edge_index[1], dtype=np.int64)
    w = np.asarray(edge_weight, dtype=np.float32)

    deg = (np.bincount(dst, weights=w.astype(np.float64), minlength=N)
           .astype(np.float32) + np.float32(1.0))
    dinv = (np.float32(1.0) / np.sqrt(deg)).astype(np.float32)
    norm = (dinv[src] * w * dinv[dst]).astype(np.float32)

    # synthesized self-loop edges with weight dinv^2
    src_all = np.concatenate([src, np.arange(N, dtype=np.int64)])
    dst_all = np.concatenate([dst, np.arange(N, dtype=np.int64)])
    norm_all = np.concatenate([norm, (dinv * dinv).astype(np.float32)])

    core = dst_all // SHARD
    dloc = dst_all - core * SHARD
    block = dloc // P
    dib = dloc % P                               # dst-in-block 0..127
    half = (src_all >= HALF).astype(np.int64)
    idx_rel = (src_all - half * HALF).astype(np.int16)

    # order by (core, block, half), then by dst-in-block within each bucket
    key = (core * BLOCKS + block) * 2 + half
    order = np.lexsort((dib, key))
    s_key = key[order]
    s_idx = idx_rel[order]
    s_dib = dib[order]
    s_norm = norm_all[order]

    counts = np.bincount(s_key, minlength=CORES * BLOCKS * 2).reshape(CORES, BLOCKS, 2)
    Tb = (counts.max(axis=0) + P - 1) // P       # [BLOCKS, 2] tiles per block-half

    # slot layout: per gather-group g: [h0 of g's blocks..., h1 of g's blocks...]
    groups = [list(range(g, min(g + GROUP_BLOCKS, BLOCKS)))
              for g in range(0, BLOCKS, GROUP_BLOCKS)]
    slot_off = np.zeros((BLOCKS, 2), dtype=np.int64)
    off = 0
    for g in groups:
        for h in (0, 1):
            for b in g:
                slot_off[b, h] = off
                off += Tb[b, h] * P
    TOT = off
    T_TOTAL = TOT // P

    grp_start = np.zeros(CORES * BLOCKS * 2 + 1, dtype=np.int64)
    np.cumsum(np.bincount(s_key, minlength=CORES * BLOCKS * 2), out=grp_start[1:])
    rank = np.arange(len(s_key)) - grp_start[s_key]
    b_of = (s_key // 2) % BLOCKS
    h_of = s_key % 2
    slot = slot_off[b_of, h_of] + rank
    c_of = s_key // (BLOCKS * 2)

    idx_flat = np.zeros((CORES, TOT), dtype=np.int16)
    dib_flat = np.zeros((CORES, TOT), dtype=np.int64)
    norm_flat = np.zeros((CORES, TOT), dtype=np.float32)
    valid = np.zeros((CORES, TOT), dtype=bool)
    idx_flat[c_of, slot] = s_idx
    dib_flat[c_of, slot] = s_dib
    norm_flat[c_of, slot] = s_norm
    valid[c_of, slot] = True

    # per-tile dst range union over cores (real edges only)
    dib_t = dib_flat.reshape(CORES, T_TOTAL, P)
    val_t = valid.reshape(CORES, T_TOTAL, P)
    tmin = np.where(val_t, dib_t, 10**6).min(axis=(0, 2))      # [T_TOTAL]
    tmax = np.where(val_t, dib_t, -1).max(axis=(0, 2))
    empty = tmax < 0
    tmin[empty] = 0
    tmax[empty] = 0

    # first issued tile per block must be full-width (start=True covers psum)
    tile_full = np.zeros(T_TOTAL, dtype=bool)
    for b in range(BLOCKS):
        if Tb[b, 0] > 0:
            tile_full[slot_off[b, 0] // P] = True
        elif Tb[b, 1] > 0:
            tile_full[slot_off[b, 1] // P] = True
    tile_full |= (tmax - tmin) >= WNARROW
    w0 = np.minimum(tmin, P - WNARROW)
    w0[tile_full] = 0
    full_idx = np.cumsum(tile_full) - 1          # position in compacted full list
    NFULL = int(tile_full.sum())

    # dstN: window-relative dst for narrow tiles (full tiles unused there)
    w0_of_slot = w0[np.arange(TOT) // P]
    dstn_flat = (dib_flat - w0_of_slot[None, :]).astype(np.float32)
    dstn_flat[~valid] = -1.0                      # pad slots match nothing
    dibF_flat = dib_flat.astype(np.float32)
    dibF_flat[~valid] = -1.0

    idx_wrapped = np.ascontiguousarray(
        np.tile(idx_flat.reshape(CORES, -1, 16).transpose(0, 2, 1), (1, 8, 1)))
    dstN = np.ascontiguousarray(dstn_flat.reshape(CORES, T_TOTAL, P).transpose(0, 2, 1))
    normF = np.ascontiguousarray(norm_flat.reshape(CORES, T_TOTAL, P).transpose(0, 2, 1))
    # compacted full-tile tables
    ft = np.flatnonzero(tile_full)
    dstFull = np.ascontiguousarray(
        dibF_flat.reshape(CORES, T_TOTAL, P).transpose(0, 2, 1)[:, :, ft])
    normFull = np.ascontiguousarray(
        norm_flat.reshape(CORES, T_TOTAL, P).transpose(0, 2, 1)[:, :, ft])

    # trailing-pad skip: for each (group, half) gather, the LAST bucket's pad
    # slots are trailing in the gather's idx list; mark them -1 (not gathered)
    # and record per-core valid counts. Groups 0/1 keep full pads (first-touch
    # warms the G pool slots so skipped slots never expose uninitialized SBUF).
    NG2 = len(groups) * 2
    cnt_tab = np.zeros((CORES, NG2), np.int32)
    idx_flat2 = idx_wrapped  # already built; we patch idx_flat below and rebuild
    for gi, g in enumerate(groups):
        for h in (0, 1):
            j = gi * 2 + h
            first_b = g[0]
            last_b = g[-1]
            pre = sum(int(Tb[b, h]) * P for b in g[:-1])
            vc = pre + counts[:, last_b, h]              # [CORES]
            padded = pre + int(Tb[last_b, h]) * P
            if gi >= 2:
                for c in range(CORES):
                    s0 = int(slot_off[last_b, h]) + int(counts[c, last_b, h])
                    s1 = int(slot_off[last_b, h]) + int(Tb[last_b, h]) * P
                    idx_flat[c, s0:s1] = -1
                    if vc[c] == 0 and padded > 0:
                        idx_flat[c, int(slot_off[first_b, h])] = 0
                        vc[c] = 1
            else:
                vc[:] = padded
            cnt_tab[:, j] = vc
    idx_wrapped = np.ascontiguousarray(
        np.tile(idx_flat.reshape(CORES, -1, 16).transpose(0, 2, 1), (1, 8, 1)))

    pl = Plan()
    pl.groups, pl.Tb, pl.slot_off, pl.T_TOTAL = groups, Tb, slot_off, T_TOTAL
    pl.tile_full, pl.w0, pl.full_idx, pl.NFULL = tile_full, w0, full_idx, NFULL
    pl.NG2 = NG2
    return pl, idx_wrapped, dstN, normF, dstFull, normFull, cnt_tab


def _build_program(pl, skip=(), hzero=True):
    import concourse.bass as bass
    import concourse.tile as tile
    import concourse.mybir as mybir
    from concourse import library_config

    groups, Tb, slot_off, T_TOTAL = pl.groups, pl.Tb, pl.slot_off, pl.T_TOTAL
    tile_full, w0, full_idx, NFULL = pl.tile_full, pl.w0, pl.full_idx, pl.NFULL
    full_pos = np.flatnonzero(tile_full)
    HZERO = bool(hzero)

    f32 = mybir.dt.float32
    NCH_ = (BLOCKS + 3) // 4
    PPAD = ((NCH_ + 1) // 2) * 512
    nc = bass.Bass("TRN2", target_bir_lowering=False, debug=False, num_devices=CORES)

    x_d = nc.dram_tensor("x", [N, IN_C], f32, kind="ExternalInput")
    idx_d = nc.dram_tensor("idx", [P, (T_TOTAL * P) // 16], mybir.dt.int16, kind="ExternalInput")
    dstn_d = nc.dram_tensor("dstn", [P, T_TOTAL], f32, kind="ExternalInput")
    normf_d = nc.dram_tensor("normf", [P, T_TOTAL], f32, kind="ExternalInput")
    dstfull_d = nc.dram_tensor("dstfull", [P, max(NFULL, 1)], f32, kind="ExternalInput")
    normfull_d = nc.dram_tensor("normfull", [P, max(NFULL, 1)], f32, kind="ExternalInput")
    iota_d = nc.dram_tensor("iota", [P, P], f32, kind="ExternalInput")
    cnt_d = nc.dram_tensor("cnt", [1, pl.NG2], mybir.dt.int32, kind="ExternalInput")
    wgcnT_d = nc.dram_tensor("wgcnT", [IN_C, HID], f32, kind="ExternalInput")
    wihT_d = nc.dram_tensor("wihT", [P, 3 * HID], f32, kind="ExternalInput")
    whhT_d = nc.dram_tensor("whhT", [P, 3 * HID], f32, kind="ExternalInput")
    br_d = nc.dram_tensor("br", [P, 1], f32, kind="ExternalInput")
    bz_d = nc.dram_tensor("bz", [P, 1], f32, kind="ExternalInput")
    bihn_d = nc.dram_tensor("bihn", [P, 1], f32, kind="ExternalInput")
    bhhn_d = nc.dram_tensor("bhhn", [P, 1], f32, kind="ExternalInput")
    hmemT_d = nc.dram_tensor("hmemT", [P, PPAD], f32, kind="ExternalInput")
    out_d = nc.dram_tensor("outT", [HID, PADC], f32, kind="ExternalOutput")

    with tile.TileContext(nc, trace_sim=False) as tc:
        nc.gpsimd.load_library(library_config.mlp)
        with (
            tc.tile_pool(name="const", bufs=1) as cpool,
            tc.tile_pool(name="agg", bufs=1) as apool,
            tc.tile_pool(name="g0", bufs=2) as g0pool,
            tc.tile_pool(name="g1", bufs=2) as g1pool,
            tc.tile_pool(name="s", bufs=2) as spool,
            tc.tile_pool(name="sf", bufs=2) as sfpool,
            tc.tile_pool(name="ps1", bufs=(3 if hzero else 2), space="PSUM") as ppool,
            tc.tile_pool(name="p2", bufs=2) as sb2,
            tc.tile_pool(name="ps2g", bufs=2, space="PSUM") as pp2g,
            tc.tile_pool(name="ps2", bufs=1, space="PSUM") as pp2,
        ):
            idx_t = cpool.tile([P, (T_TOTAL * P) // 16], mybir.dt.int16)
            dstn_t = cpool.tile([P, T_TOTAL], f32)
            norm_t = cpool.tile([P, T_TOTAL], f32)
            dstfull_t = cpool.tile([P, max(NFULL, 1)], f32)
            normfull_t = cpool.tile([P, max(NFULL, 1)], f32)
            iota_t = cpool.tile([P, P], f32)
            cnt_t = cpool.tile([1, pl.NG2], mybir.dt.int32)
            wgcnT_t = cpool.tile([IN_C, HID], f32)
            wihT_t = cpool.tile([P, 3 * HID], f32)
            whhT_t = cpool.tile([P, 3 * HID], f32)
            br_t = cpool.tile([P, 1], f32)
            bz_t = cpool.tile([P, 1], f32)
            bihn_t = cpool.tile([P, 1], f32)
            bhhn_t = cpool.tile([P, 1], f32)
            hmemT_t = cpool.tile([P, PPAD], f32)
            loads = [(idx_t, idx_d), (dstn_t, dstn_d), (norm_t, normf_d),
                     (dstfull_t, dstfull_d), (normfull_t, normfull_d),
                     (iota_t, iota_d), (cnt_t, cnt_d), (wgcnT_t, wgcnT_d), (wihT_t, wihT_d),
                     (br_t, br_d), (bz_t, bz_d), (bihn_t, bihn_d), (bhhn_t, bhhn_d)]
            if not hzero:
                loads += [(whhT_t, whhT_d), (hmemT_t, hmemT_d)]
            for t, d in loads:
                nc.sync.dma_start(out=t[:], in_=d[:])

            NCH = (BLOCKS + 3) // 4            # 512-wide gcn/gru chunks
            agg_tiles = []
            for c in range(NCH):
                a_t = apool.tile([P, min(512, PADC - c * 512)], f32, name=f"agg{c}")
                agg_tiles.append(a_t)

            # ---- phase 1: gather + scatter-matmul per group ----
            if True:
                iota3n = iota_t[:, 0:WNARROW].rearrange("p (a j) -> p a j", a=1)
                iota3f = iota_t[:].rearrange("p (a j) -> p a j", a=1)
                cnt_reg = nc.gpsimd.alloc_register("gather_cnt")

                def _creg(j):
                    nc.gpsimd.reg_load(cnt_reg, cnt_t[0:1, j:j + 1])
                    return cnt_reg

                def do_group(g):
                    gi = groups.index(g)
                    t0 = int(slot_off[g[0], 0]) // P
                    Tg0 = sum(int(Tb[b, 0]) for b in g)
                    Tg1 = sum(int(Tb[b, 1]) for b in g)
                    Tg = Tg0 + Tg1
                    f0 = int(np.searchsorted(full_pos, t0))
                    nf_g = int(tile_full[t0:t0 + Tg].sum())

                    gh = [None, None]
                    if Tg0 and 'gather' not in skip:
                        g_t0 = g0pool.tile([P, Tg0, IN_C], f32, tag="g0")
                        gh[0] = g_t0
                        nc.gpsimd.dma_gather(
                            g_t0[:], x_d[0:HALF, :],
                            idx_t[:, t0 * 8:(t0 + Tg0) * 8],
                            Tg0 * P, _creg(gi * 2 + 0), IN_C, single_packet=False)
                    if Tg1 and 'gather' not in skip:
                        g_t1 = g1pool.tile([P, Tg1, IN_C], f32, tag="g1")
                        gh[1] = g_t1
                        nc.gpsimd.dma_gather(
                            g_t1[:], x_d[HALF:N, :],
                            idx_t[:, (t0 + Tg0) * 8:(t0 + Tg) * 8],
                            Tg1 * P, _creg(gi * 2 + 1), IN_C, single_packet=False)

                    if 'sbuild' in skip:
                        return
                    s_t = spool.tile([P, Tg, WNARROW], f32, tag="s")
                    nc.vector.tensor_tensor(
                        out=s_t[:],
                        in0=iota3n.to_broadcast([P, Tg, WNARROW]),
                        in1=dstn_t[:, t0:t0 + Tg].to_broadcast([P, Tg, WNARROW]),
                        op=mybir.AluOpType.is_equal)
                    nc.vector.tensor_tensor(
                        out=s_t[:], in0=s_t[:],
                        in1=norm_t[:, t0:t0 + Tg].to_broadcast([P, Tg, WNARROW]),
                        op=mybir.AluOpType.mult)
                    sf_t = None
                    if nf_g:
                        sf_t = sfpool.tile([P, nf_g, P], f32, tag="sf")
                        nc.vector.tensor_tensor(
                            out=sf_t[:],
                            in0=iota3f.to_broadcast([P, nf_g, P]),
                            in1=dstfull_t[:, f0:f0 + nf_g].to_broadcast([P, nf_g, P]),
                            op=mybir.AluOpType.is_equal)
                        nc.vector.tensor_tensor(
                            out=sf_t[:], in0=sf_t[:],
                            in1=normfull_t[:, f0:f0 + nf_g].to_broadcast([P, nf_g, P]),
                            op=mybir.AluOpType.mult)

                    if 'mm' in skip or 'gather' in skip:
                        return
                    for b in g:
                        nmm = int(Tb[b, 0] + Tb[b, 1])
                        if nmm == 0:
                            continue
                        psum_t = ppool.tile([P, P], f32, space="PSUM", tag="ps")
                        k = 0
                        for h in (0, 1):
                            gt0 = int(slot_off[b, h]) // P        # global tile base
                            rel_g = gt0 - t0 - (Tg0 if h else 0)  # within gh[h]
                            for t in range(int(Tb[b, h])):
                                ti = gt0 + t
                                if tile_full[ti]:
                                    rhs = sf_t[:, int(full_idx[ti]) - f0, :]
                                    out_ap = psum_t[:]
                                else:
                                    rhs = s_t[:, ti - t0, :]
                                    ws = int(w0[ti])
                                    out_ap = psum_t[:, ws:ws + WNARROW]
                                nc.tensor.matmul(
                                    out=out_ap,
                                    lhsT=gh[h][:, rel_g + t, :],
                                    rhs=rhs,
                                    start=(k == 0), stop=(k == nmm - 1),
                                    skip_group_check=True)
                                k += 1
                        nc.scalar.copy(
                            out=agg_tiles[b // 4][:, (b % 4) * P:(b % 4 + 1) * P],
                            in_=psum_t[:])

            # ---- phase 2: GCN linear + GRU, feature-major, chunks of 512 ----
            AF = mybir.ActivationFunctionType

            def do_pair(p):
                c0, c1 = 2 * p, 2 * p + 1
                pair = [c for c in (c0, c1) if c < NCH]
                w = [min(512, PADC - c * 512) for c in pair]
                cw = max(w)
                ph = len(pair) * HID

                gcn_ps = pp2g.tile([P, cw], f32, space="PSUM", tag="gcn")
                for i, c in enumerate(pair):
                    nc.tensor.matmul(out=gcn_ps[i * HID:(i + 1) * HID, 0:w[i]],
                                     lhsT=wgcnT_t[:], rhs=agg_tiles[c][:, 0:w[i]],
                                     start=True, stop=True)
                gcn_sb = sb2.tile([P, cw], f32, tag="gcnsb")
                nc.scalar.copy(out=gcn_sb[0:ph, 0:cw], in_=gcn_ps[0:ph, 0:cw])

                def gate_mm(tag, wslice):
                    ps = pp2.tile([P, cw], f32, space="PSUM", tag=tag)
                    for i, c in enumerate(pair):
                        hh = slice(i * HID, (i + 1) * HID)
                        nc.tensor.matmul(out=ps[hh, 0:w[i]], lhsT=wihT_t[hh, wslice],
                                         rhs=gcn_sb[hh, 0:w[i]],
                                         start=True, stop=HZERO)
                        if not HZERO:
                            nc.tensor.matmul(out=ps[hh, 0:w[i]], lhsT=whhT_t[hh, wslice],
                                             rhs=hmemT_t[hh, p * 512:p * 512 + w[i]],
                                             start=False, stop=True)
                    return ps

                r_ps = gate_mm("r", slice(0, HID))
                z_ps = gate_mm("z", slice(HID, 2 * HID))
                n_ps = pp2.tile([P, cw], f32, space="PSUM", tag="n")
                for i, c in enumerate(pair):
                    hh = slice(i * HID, (i + 1) * HID)
                    nc.tensor.matmul(out=n_ps[hh, 0:w[i]], lhsT=wihT_t[hh, 2 * HID:3 * HID],
                                     rhs=gcn_sb[hh, 0:w[i]], start=True, stop=True)

                r_sb = sb2.tile([P, cw], f32, tag="r_sb")
                nc.scalar.activation(out=r_sb[0:ph, 0:cw], in_=r_ps[0:ph, 0:cw],
                                     func=AF.Sigmoid, bias=br_t[0:ph, :])
                z_sb = sb2.tile([P, cw], f32, tag="z_sb")
                nc.scalar.activation(out=z_sb[0:ph, 0:cw], in_=z_ps[0:ph, 0:cw],
                                     func=AF.Sigmoid, bias=bz_t[0:ph, :])

                rhn = sb2.tile([P, cw], f32, tag="rhn")
                if HZERO:
                    # h_n == b_hhn (constant per partition)
                    nc.vector.tensor_scalar(out=rhn[0:ph, 0:cw], in0=r_sb[0:ph, 0:cw],
                                            scalar1=bhhn_t[0:ph, :], scalar2=None,
                                            op0=mybir.AluOpType.mult)
                else:
                    hn_ps = pp2.tile([P, cw], f32, space="PSUM", tag="hn")
                    for i, c in enumerate(pair):
                        hh = slice(i * HID, (i + 1) * HID)
                        nc.tensor.matmul(out=hn_ps[hh, 0:w[i]],
                                         lhsT=whhT_t[hh, 2 * HID:3 * HID],
                                         rhs=hmemT_t[hh, p * 512:p * 512 + w[i]],
                                         start=True, stop=True)
                    hn_sb = sb2.tile([P, cw], f32, tag="hn_sb")
                    nc.scalar.activation(out=hn_sb[0:ph, 0:cw], in_=hn_ps[0:ph, 0:cw],
                                         func=AF.Identity, bias=bhhn_t[0:ph, :])
                    nc.vector.tensor_mul(out=rhn[0:ph, 0:cw], in0=r_sb[0:ph, 0:cw],
                                         in1=hn_sb[0:ph, 0:cw])

                pre = sb2.tile([P, cw], f32, tag="pre")
                nc.vector.tensor_add(out=pre[0:ph, 0:cw], in0=rhn[0:ph, 0:cw], in1=n_ps[0:ph, 0:cw])
                nact = sb2.tile([P, cw], f32, tag="nact")
                nc.scalar.activation(out=nact[0:ph, 0:cw], in_=pre[0:ph, 0:cw],
                                     func=AF.Tanh, bias=bihn_t[0:ph, :])

                h_sb = sb2.tile([P, cw], f32, tag="h_sb")
                if HZERO:
                    zn = sb2.tile([P, cw], f32, tag="zn")
                    nc.vector.tensor_mul(out=zn[0:ph, 0:cw], in0=z_sb[0:ph, 0:cw], in1=nact[0:ph, 0:cw])
                    nc.vector.tensor_sub(out=h_sb[0:ph, 0:cw], in0=nact[0:ph, 0:cw], in1=zn[0:ph, 0:cw])
                else:
                    d_sb = sb2.tile([P, cw], f32, tag="d_sb")
                    nc.vector.tensor_sub(out=d_sb[0:ph, 0:cw],
                                         in0=hmemT_t[0:ph, p * 512:p * 512 + cw],
                                         in1=nact[0:ph, 0:cw])
                    e_sb = sb2.tile([P, cw], f32, tag="e_sb")
                    nc.vector.tensor_mul(out=e_sb[0:ph, 0:cw], in0=z_sb[0:ph, 0:cw], in1=d_sb[0:ph, 0:cw])
                    nc.vector.tensor_add(out=h_sb[0:ph, 0:cw], in0=nact[0:ph, 0:cw], in1=e_sb[0:ph, 0:cw])
                for i, c in enumerate(pair):
                    nc.sync.dma_start(out=out_d[:, c * 512:c * 512 + w[i]],
                                      in_=h_sb[i * HID:(i + 1) * HID, 0:w[i]])

            NPAIR = (NCH + 1) // 2
            for p in range(NPAIR):
                for g in groups[4 * p:4 * p + 4]:
                    do_group(g)
                if 'phase2' not in skip and p >= 1:
                    do_pair(p - 1)
            if 'phase2' not in skip:
                do_pair(NPAIR - 1)

    return nc


def kernel(x, edge_index, edge_weight, W_gcn, b_gcn, W_ih, W_hh, b_ih, b_hh, h_mem):
    global last_nc
    import concourse.mybir as mybir
    from concourse.bass_utils import run_bass_kernel_spmd

    x = np.ascontiguousarray(np.asarray(x, dtype=np.float32))
    h_mem = np.asarray(h_mem, dtype=np.float32)
    W_gcn = np.asarray(W_gcn, dtype=np.float32)
    W_ih = np.asarray(W_ih, dtype=np.float32)
    W_hh = np.asarray(W_hh, dtype=np.float32)
    b_gcn = np.asarray(b_gcn, dtype=np.float32)
    b_ih = np.asarray(b_ih, dtype=np.float32)
    b_hh = np.asarray(b_hh, dtype=np.float32)

    pl, idx_wrapped, dstN, normF, dstFull, normFull, cnt_tab = _host_prep(
        x, edge_index, edge_weight)

    hzero = not np.any(h_mem)
    nc = _build_program(pl, hzero=hzero)
    last_nc = nc

    from bir_fixups import split_sync_waits
    mybir.codegen_inst_isa_subclasses(nc)
    split_sync_waits(nc, max_waits=1)

    b_ihp = (b_ih + W_ih @ b_gcn).astype(np.float32)
    br = np.tile((b_ihp[0:HID] + b_hh[0:HID]).astype(np.float32), 2).reshape(P, 1)
    bz = np.tile((b_ihp[HID:2 * HID] + b_hh[HID:2 * HID]).astype(np.float32), 2).reshape(P, 1)
    bihn = np.tile(b_ihp[2 * HID:3 * HID].astype(np.float32), 2).reshape(P, 1)
    bhhn = np.tile(b_hh[2 * HID:3 * HID].astype(np.float32), 2).reshape(P, 1)

    iota_np = np.broadcast_to(np.arange(P, dtype=np.float32), (P, P)).copy()
    wgcnT = np.ascontiguousarray(W_gcn.T)
    wihT = np.ascontiguousarray(np.vstack([W_ih.T, W_ih.T]))
    whhT = np.ascontiguousarray(np.vstack([W_hh.T, W_hh.T]))

    hmemT_flat = np.zeros((CORES, HID, PADC), np.float32)
    hmemT_flat[:, :, 0:SHARD] = h_mem.reshape(CORES, SHARD, HID).transpose(0, 2, 1)
    NCH = (BLOCKS + 3) // 4
    NPAIR = (NCH + 1) // 2
    hmemT = np.zeros((CORES, P, NPAIR * 512), np.float32)
    for c in range(NCH):
        w = min(512, PADC - c * 512)
        pcol = (c // 2) * 512
        hmemT[:, (c % 2) * HID:(c % 2 + 1) * HID, pcol:pcol + w] = \
            hmemT_flat[:, :, c * 512:c * 512 + w]

    in_maps = []
    for c in range(CORES):
        in_maps.append({
            "x": x, "idx": idx_wrapped[c], "dstn": dstN[c], "normf": normF[c],
            "dstfull": dstFull[c], "normfull": normFull[c],
            "cnt": cnt_tab[c:c + 1, :],
            "iota": iota_np, "wgcnT": wgcnT, "wihT": wihT, "whhT": whhT,
            "br": br, "bz": bz, "bihn": bihn, "bhhn": bhhn, "hmemT": hmemT[c],
        })

    res = run_bass_kernel_spmd(nc, in_maps, core_ids=list(range(CORES)))
    out = np.empty((N, HID), np.float32)
    for c in range(CORES):
        out[c * SHARD:(c + 1) * SHARD, :] = res.results[c]["outT"][:, 0:SHARD].T
    return out
